# revision 1
# baseline (speedup 1.0000x reference)
"""Trainium2 Bass kernel for nn_CaptionHead (segment_reduce).

Computes, for full-size inputs:
    point_feats = adapter_feats[v2p_map]            # [N_PTS, D]
    gathered    = point_feats[point_idx]            # [T, D]
    sums        = segment_sum(gathered, seg_ids, S) # [S, D]
    pooled      = l2norm(sums / max(counts, 1))     # == l2norm(sums)
    logits      = (pooled @ l2norm(ce).T) * exp(logit_scale)

Distribution: adapter_feats is sharded by voxel across the 8 cores
(25000 rows each, stored bf16 so a gathered row is one full-rate 512 B
descriptor; shard-local indices fit the int16 dma_gather path).  Host
preprocessing composes cidx = v2p_map[point_idx], routes each point to
the core owning its voxel, and DEDUPLICATES per (core, 128-seg chunk,
voxel): the gather is descriptor-count-bound on real HW (~2.3 ns/row
fixed cost), so each distinct row is fetched once and scattered with
count-weighted one-hot layers (layer L = the L-th (seg, count) pair of
a row; rows sort multiplicity-first so layers L>=2 and count>1 tiles
stay a short prefix).

On device, each core streams its rows with dma_gather (4 SWDGE queues,
a shared full-batch count register -- per-batch Q7 reg_loads were a
serial bottleneck -- and a 13-buffer pipeline; PAD INDICES MUST BE -1:
non-negative pads hang the gather ucode).  One-hot weights are built on
the DVE in batches of 8 tiles in a transposed [128, seg, tile] layout
against a pre-expanded iota so both streamed operands keep a packed
2-byte last dim (the DVE 2x fast path; a per-tile build was ~3x
slower).  Count-weighted prefix tiles use the fused TensorScalarPtr
(iota == seg) * cnt path.  Each chunk accumulates its [128, 256] PSUM
block via the one-hot matmuls, the [S, D] partial sums ReduceScatter
(f32 -- bf16 collectives hang) in two halves overlapped with the loop,
and each core normalizes its 256 segment rows (1/count cancels in the
l2norm) and multiplies against host-prenormalized caption embeddings
(bf16).  Core r returns logits rows for chunks r and 8+r; the host
concatenates the blocks.
"""

import math

import numpy as np

N_VOX = 200000
N_PTS = 500000
T_FULL = 1000000
S_FULL = 2048
D_FULL = 256
N_CORES = 8
P = 128


def _preprocess(v2p_map, point_idx, seg_ids, n_cores, vox_per_core, n_chunks, trim=True):
    """Route points to voxel-owning cores, dedup per (core, chunk, voxel), pad.

    Each distinct (core, chunk, voxel) becomes ONE gathered row (the gather is
    descriptor-count-bound on HW, so duplicate rows are pure waste).  A row
    scatters into its chunk's 128 segments through count-weighted one-hot
    layers: layer L holds the L-th (seg, count) pair of the row.  Rows are
    ordered within each chunk by pair-count descending so layer L>=2 only
    touches the first few tiles.

    Returns (idx16, layers, tiles_per_chunk, layer_tiles, counts):
      idx16[m]:  [128, NIDX//16] int16 shard-local voxel index per row in
                 dma_gather's 16-partition-wrapped, 8x-replicated layout.
      layers[m]: list over L of (segf, cntf) float32 [128, n_chunks*layer_tiles[L]]
                 chunk-local seg id / multiplicity (seg -1, cnt 0 for padding).
      counts:    [n_cores, n_chunks] valid row count per cell (for trim).
    """
    v2p = np.asarray(v2p_map).astype(np.int64)
    pidx = np.asarray(point_idx).astype(np.int64)
    seg = np.asarray(seg_ids).astype(np.int64)
    cidx = v2p[pidx]                      # composed voxel index per point
    core = cidx // vox_per_core
    lvox = cidx - core * vox_per_core
    chunk = seg >> 7                      # 128 segments per chunk
    segl = seg & 127
    cell = core * n_chunks + chunk        # [0, n_cores*n_chunks)
    key3 = (cell * vox_per_core + lvox) * 128 + segl
    uk, ucnt = np.unique(key3, return_counts=True)      # sorted pairs
    rowkey = uk // 128                                  # (cell, voxel)
    useg = (uk % 128).astype(np.float32)
    urow, first_idx, row_inv, npairs = np.unique(
        rowkey, return_index=True, return_inverse=True, return_counts=True
    )
    pair_layer = np.arange(len(uk)) - first_idx[row_inv]  # rank within row
    rcell = (urow // vox_per_core).astype(np.int64)
    rvox = (urow % vox_per_core).astype(np.int64)
    # rows with any multiplicity>1 pair first (their one-hots need the
    # slower fused count-scale op), then by pair count so layer L>=2 stays a
    # small tile prefix, then voxel for determinism
    has2 = np.zeros(len(urow), bool)
    np.logical_or.at(has2, row_inv, ucnt > 1)
    order = np.lexsort((rvox, -npairs, ~has2, rcell))
    counts = np.bincount(rcell, minlength=n_cores * n_chunks)
    tiles_per_chunk = max(1, math.ceil(counts.max() / P))
    offs = np.concatenate([[0], np.cumsum(counts)])
    pos_sorted = np.arange(len(urow)) - offs[rcell[order]]
    pos = np.empty(len(urow), np.int64)
    pos[order] = pos_sorted                              # in-cell row slot
    max_layers = int(npairs.max())
    # layer_tiles[L]: tiles covering every pair at layer L (from actual max
    # slot); cnt_tiles[L]: tile prefix containing every multiplicity>1 pair
    pr_cell0 = rcell[row_inv]
    pr_pos0 = pos[row_inv]
    layer_tiles = []
    cnt_tiles = []
    for L in range(max_layers):
        m = pair_layer == L
        layer_tiles.append(int(pr_pos0[m].max()) // P + 1)
        m2 = m & (ucnt > 1)
        cnt_tiles.append(int(pr_pos0[m2].max()) // P + 1 if m2.any() else 0)
    npc = tiles_per_chunk * P
    vox_arr = np.full((n_cores, n_chunks, npc), -1 if trim else 0, np.int16)
    vox_arr.reshape(n_cores * n_chunks, npc)[rcell, pos] = rvox.astype(np.int16)
    seg_arrs, cnt_arrs = [], []
    pr_cell = pr_cell0                                   # per-pair cell
    pr_pos = pr_pos0                                     # per-pair row slot
    for L in range(max_layers):
        w = layer_tiles[L] * P
        sa = np.full((n_cores, n_chunks, w), -1.0, np.float32)
        ca = np.zeros((n_cores, n_chunks, w), np.float32)
        m = pair_layer == L
        sa.reshape(n_cores * n_chunks, w)[pr_cell[m], pr_pos[m]] = useg[m]
        ca.reshape(n_cores * n_chunks, w)[pr_cell[m], pr_pos[m]] = ucnt[m]
        seg_arrs.append(sa)
        cnt_arrs.append(ca)

    idx16 = []
    layers = []
    for m in range(n_cores):
        arr = vox_arr[m].reshape(-1, 16).T          # [16, NIDX//16]
        idx16.append(np.ascontiguousarray(np.tile(arr, (8, 1))))
        layers.append([
            (np.ascontiguousarray(seg_arrs[L][m].reshape(-1, P).T),
             np.ascontiguousarray(cnt_arrs[L][m].reshape(-1, P).T))
            for L in range(max_layers)
        ])
    return (idx16, layers, tiles_per_chunk, layer_tiles, cnt_tiles,
            counts.reshape(n_cores, n_chunks))


def _build_nc(tiles_per_chunk, vox_per_core, D, S, n_cores, layer_tiles,
              cnt_tiles=None, batch_tiles=8, main_reps=1, mode="full",
              single_core=False, gp_bufs=13, oh_bufs=8, acc_bufs=4,
              full_batches=None, need_memset=False):
    """mode: "full" | "nomm" (gathers only) | "nogather" (compute only)
    | "noonehot" (gather + matmul, constant weights).  main_reps repeats the
    main loop; with mode="full" the output stays correct (each rep recomputes
    the same sums; only the last is copied out)."""
    import concourse.bacc as bacc
    import concourse.mybir as mybir
    import concourse.tile as tile
    from concourse.masks import make_identity

    f32 = mybir.dt.float32
    bf16 = mybir.dt.bfloat16
    i16 = mybir.dt.int16
    i32 = mybir.dt.int32
    n_chunks = S // P
    NT = n_chunks * tiles_per_chunk            # total point tiles
    NIDX = NT * P                              # total gathered rows
    out_rows = S // n_cores                    # 256
    blk_tiles = out_rows // P                  # 2
    k_tiles = D // P                           # 2
    n_cols = 512                               # moving-operand tile width
    n_tiles_out = S // n_cols                  # 4

    nc = bacc.Bacc(
        "TRN2",
        target_bir_lowering=False,
        debug=False,
        enable_asserts=False,
        num_devices=n_cores,
        # SWDGE descriptor-ring carveout: must hold two in-flight
        # dma_gathers of batch_tiles*128 descriptors each.
        dynamic_dma_scratch_size=32768,
        # round-robin gathers over all 4 SWDGE queues: each queue's
        # descriptor generation runs on its own Q7 core pair.
        num_swdge_queues=4,
    )

    # adapter rows are plain bf16: 512 B gathered per point, which both
    # halves HBM gather traffic vs an f32/hi-lo row and stays exactly at the
    # DMA full-rate descriptor size (>= 512 B).  Precision: the one-hot
    # matmul accumulates bf16 values in f32 PSUM; per-logit error lands
    # ~1e-3 relative, far under the 2e-2 gate.
    adapter = nc.dram_tensor("adapter", [vox_per_core, D], bf16, kind="ExternalInput")
    idx16 = nc.dram_tensor("idx16", [P, NIDX // 16], i16, kind="ExternalInput")
    # per-layer (seg, count) pairs, concatenated along columns; layer L
    # occupies cols [layer_off[L], layer_off[L+1]) with n_chunks*layer_tiles[L]
    # columns (tile-major within each layer).
    layer_off = [0]
    for lt in layer_tiles:
        layer_off.append(layer_off[-1] + n_chunks * lt)
    if cnt_tiles is None:
        cnt_tiles = [lt for lt in layer_tiles]  # all tiles use the fused op
    segf = nc.dram_tensor("segf", [P, layer_off[-1]], f32, kind="ExternalInput")
    cntf = nc.dram_tensor("cntf", [P, layer_off[-1]], f32, kind="ExternalInput")
    # bf16 copy of segf for the batched transposed one-hot builds, and the
    # batch_tiles-fold expanded iota ([P, j, b] -> j) whose packed last dim
    # keeps the DVE 2x fast path on both streamed operands
    segfb = nc.dram_tensor("segfb", [P, layer_off[-1]], bf16, kind="ExternalInput")
    iota = nc.dram_tensor("iota", [P, P], bf16, kind="ExternalInput")
    iotax = nc.dram_tensor("iotax", [P, P * batch_tiles], bf16, kind="ExternalInput")
    # caption embeds arrive L2-normalized from the host; only the transposed
    # copy is needed for the logits matmul.
    cet = nc.dram_tensor("cet", [D, S], bf16, kind="ExternalInput")
    lsr = nc.dram_tensor("lsr", [P, 1], f32, kind="ExternalInput")
    n_batches = (tiles_per_chunk + batch_tiles - 1) // batch_tiles
    if full_batches is None:
        full_batches = [False] * (n_chunks * n_batches)
    cnts = nc.dram_tensor("cnts", [1, n_chunks * n_batches], i32, kind="ExternalInput")
    out = nc.dram_tensor("logits_block", [out_rows, S], f32, kind="ExternalOutput")
    cc_in = nc.dram_tensor("cc_in", [S, D], f32, kind="Internal")
    half_rows = S // 2
    cc_out_h = [
        nc.dram_tensor(f"cc_out{h}", [half_rows // n_cores, D], f32, kind="Internal")
        for h in range(2)
    ]

    with tile.TileContext(nc) as tc:
        with (
            tc.tile_pool(name="const", bufs=1) as constp,
            tc.tile_pool(name="gather", bufs=gp_bufs) as gp,
            tc.tile_pool(name="oh", bufs=oh_bufs) as ohp,
            tc.tile_pool(name="ohb", bufs=8) as ohbp,
            tc.tile_pool(name="misc", bufs=1) as miscp,
            tc.tile_pool(name="fin", bufs=1) as finp,
            tc.tile_pool(name="fpsum", bufs=1, space="PSUM") as fpp,
        ):
            # ---- prologue loads, shortest-critical-path first ----
            # SP (sync) queue: batch counts + the first idx stripe unblock
            # the first gather within ~2 us; segf/iota unblock the one-hots.
            cnt_sb = constp.tile([1, n_chunks * n_batches], i32)
            nc.sync.dma_start(cnt_sb[:], cnts.ap())
            # idx stripes are separate tiles so a gather only depends on the
            # stripe that covers its chunk (dep tracking is per-tile).
            chunk_cols = tiles_per_chunk * P // 16
            stripe_chunks = max(1, 2048 // chunk_cols)
            stripe_cols = stripe_chunks * chunk_cols
            stripe_bounds = []
            for s0 in range(0, NIDX // 16, stripe_cols):
                stripe_bounds.append((s0, min(s0 + stripe_cols, NIDX // 16)))
            idx_parts = [
                constp.tile([P, s1 - s0], i16, name=f"idx{s0}")
                for s0, s1 in stripe_bounds
            ]
            nc.sync.dma_start(idx_parts[0][:], idx16.ap()[:, : stripe_bounds[0][1]])
            iota_sb = constp.tile([P, P], bf16)
            nc.sync.dma_start(iota_sb[:], iota.ap())
            segf_sb = constp.tile([P, layer_off[-1]], f32)
            nc.sync.dma_start(segf_sb[:], segf.ap())
            cntf_sb = constp.tile([P, layer_off[-1]], f32)
            nc.sync.dma_start(cntf_sb[:], cntf.ap())
            segfb_sb = constp.tile([P, layer_off[-1]], bf16)
            nc.sync.dma_start(segfb_sb[:], segfb.ap())
            iotax_sb = constp.tile([P, P * batch_tiles], bf16)
            nc.sync.dma_start(iotax_sb[:], iotax.ap())
            ls_sb = finp.tile([P, 1], f32)
            nc.sync.dma_start(ls_sb[:], lsr.ap())
            # Later stripes aren't needed until chunk 4+; hint them behind
            # the first gathers so they don't hog the DMA engines up front.
            with tc.tile_wait_until(0.02):
                for (s0, s1), part in zip(stripe_bounds[1:], idx_parts[1:]):
                    nc.sync.dma_start(part[:], idx16.ap()[:, s0:s1])
            els = finp.tile([P, 1], f32)
            nc.scalar.activation(els[:], ls_sb[:], mybir.ActivationFunctionType.Exp)
            ident = constp.tile([P, P], f32)
            make_identity(nc, ident[:])
            ident_bf = constp.tile([P, P], bf16)
            nc.vector.tensor_copy(out=ident_bf[:], in_=ident[:])

            sums_sb = miscp.tile([P, n_chunks * D], f32)
            sq_scr = finp.tile([P, D], f32)

            # ACT queue: the transposed caption embeds (finale-only input).
            cet_sb = [finp.tile([P, S], bf16, tag=f"cet{k}", name=f"cet{k}")
                      for k in range(k_tiles)]
            for k in range(k_tiles):
                nc.scalar.dma_start(cet_sb[k][:], cet.ap()[k * P : (k + 1) * P, :])

            # ---- main: gather + one-hot matmul segment reduction ----
            # Chunk-staged pipeline: all of chunk c's rows are gathered into
            # one chunk-wide buffer (2 in flight), then its one-hot matmuls
            # run as a single burst.  Keeps the PE busy in solid stretches
            # (no per-batch matmul/gather coupling) and lets gathers stream
            # at full descriptor rate.
            if need_memset:
                for _slot in range(gp_bufs):
                    g_init = gp.tile([P, batch_tiles, D], bf16, tag="g",
                                     name="g_init")
                    nc.vector.memset(g_init[:], 0)
            g_static = None
            if mode == "nogather":
                g_static = miscp.tile([P, batch_tiles, D], bf16)
                nc.vector.memset(g_static[:], 1.0)
            if mode == "nomm":
                nc.vector.memset(sums_sb[:], 1.0)
            # shared register holding the full batch count: only batches that
            # are partial on some core pay a per-batch reg_load.
            vreg_full = None
            if any(full_batches):
                vreg_full = nc.gpsimd.alloc_register()
                nc.gpsimd.reg_mov(vreg_full, batch_tiles * P)

            with tc.tile_pool(name="acc", bufs=acc_bufs, space="PSUM") as accp:
                for rep in range(main_reps):
                    for c in range(n_chunks):
                        gtiles = []
                        acc = None
                        if mode != "nomm":
                            acc = accp.tile([P, D], f32, tag="acc", name="acc")
                        done = 0
                        while done < tiles_per_chunk:
                            bt = min(batch_tiles, tiles_per_chunk - done)
                            if mode == "nogather":
                                done += bt
                                continue
                            g = gp.tile([P, batch_tiles, D], bf16,
                                        tag="g", name="g")
                            gtiles.append(g)
                            col0 = (c * tiles_per_chunk + done) * P // 16
                            nidx = bt * P
                            bidx = c * n_batches + done // batch_tiles
                            part = idx_parts[col0 // stripe_cols]
                            pc0 = col0 % stripe_cols
                            if full_batches[bidx] and bt == batch_tiles:
                                vreg = vreg_full
                            else:
                                vreg = nc.gpsimd.alloc_register()
                                nc.gpsimd.reg_load(
                                    vreg, cnt_sb[0:1, bidx : bidx + 1]
                                )
                            nc.gpsimd.dma_gather(
                                out_ap=g[:, :bt, :],
                                in_ap=adapter.ap(),
                                idxs_ap=part[:, pc0 : pc0 + nidx // 16],
                                num_idxs=nidx,
                                num_idxs_reg=vreg,
                                elem_size=D,
                                queue_num=bidx % 4,
                            )
                            if vreg is not vreg_full:
                                nc.gpsimd.free_register(vreg)
                            done += bt
                        if mode == "nomm":
                            continue
                        # matmuls layer-outer: each layer streams its one-hot
                        # batches sequentially; the gathered tiles stay live
                        # for the whole chunk (gp pool is sized for it).
                        n_mm = sum(layer_tiles)
                        mm_i = 0
                        for L in range(len(layer_tiles)):
                            lt = layer_tiles[L]
                            ct = min(cnt_tiles[L], lt)
                            t = 0
                            while t < lt:
                                if mode in ("noonehot", "nogather"):
                                    ohs = None
                                    w = min(batch_tiles, lt - t)
                                elif t < ct:
                                    # multiplicity>1 prefix: fused
                                    # (iota == seg) * cnt via TensorScalarPtr
                                    w = 1
                                    col = layer_off[L] + c * lt + t
                                    oh1 = ohp.tile([P, P], bf16, tag="oh",
                                                   name="oh")
                                    nc.vector.tensor_scalar(
                                        out=oh1[:],
                                        in0=iota_sb[:],
                                        scalar1=segf_sb[:, col : col + 1],
                                        scalar2=cntf_sb[:, col : col + 1],
                                        op0=mybir.AluOpType.is_equal,
                                        op1=mybir.AluOpType.mult,
                                    )
                                    ohs = [oh1[:, :]]
                                else:
                                    # batched transposed build: w tiles per
                                    # DVE op, all operands 2-byte packed
                                    w = min(batch_tiles, lt - t)
                                    col = layer_off[L] + c * lt + t
                                    ohb = ohbp.tile([P, P, batch_tiles], bf16,
                                                    tag="ohb", name="ohb")
                                    nc.vector.tensor_tensor(
                                        out=ohb[:, :, :w],
                                        in0=segfb_sb[:, col : col + w]
                                        .unsqueeze(1).to_broadcast([P, P, w]),
                                        in1=iotax_sb[:]
                                        .rearrange("p (j b) -> p j b",
                                                   b=batch_tiles)[:, :, :w],
                                        op=mybir.AluOpType.is_equal,
                                    )
                                    ohs = [ohb[:, :, j] for j in range(w)]
                                for j in range(w):
                                    tt = t + j
                                    if mode in ("noonehot", "nogather"):
                                        oh_ap = ident_bf[:, :]
                                    else:
                                        oh_ap = ohs[j]
                                    rhs = (g_static[:, 0, :]
                                           if mode == "nogather"
                                           else gtiles[tt // batch_tiles]
                                           [:, tt % batch_tiles, :])
                                    nc.tensor.matmul(
                                        acc[:],
                                        lhsT=oh_ap,
                                        rhs=rhs,
                                        start=(mm_i == 0),
                                        stop=(mm_i == n_mm - 1),
                                    )
                                    mm_i += 1
                                t += w
                        if rep == main_reps - 1:
                            if mode != "nomm":
                                nc.vector.tensor_copy(
                                    out=sums_sb[:, c * D : (c + 1) * D],
                                    in_=acc[:],
                                )
                            # stage this chunk's partial sums (ACT HWDGE queue
                            # so the SP queue stays free for other loads)
                            nc.scalar.dma_start(
                                cc_in.ap()[c * P : (c + 1) * P, :],
                                sums_sb[:, c * D : (c + 1) * D],
                            )
                            if c in (n_chunks // 2 - 1, n_chunks - 1):
                                h = 0 if c < n_chunks // 2 else 1
                                lo = h * half_rows
                                if single_core:
                                    nc.sync.dma_start(
                                        cc_out_h[h].ap(),
                                        cc_in.ap()[lo : lo + P, :],
                                    )
                                else:
                                    nc.gpsimd.collective_compute(
                                        "ReduceScatter",
                                        mybir.AluOpType.add,
                                        replica_groups=[list(range(n_cores))],
                                        ins=[cc_in.ap()[lo : lo + half_rows, :]],
                                        outs=[cc_out_h[h].ap()],
                                    )

            # ---- finale: per half-block normalize + logits rows ----
            # Pin the finale to the end of the schedule: without this the
            # tile scheduler hoists it into the middle of the main loop
            # (its collective input *can* be ready early), where it
            # head-of-line blocks the PE/DVE queues and stalls the gather
            # buffer recycling.
            finale_ctx = tc.tile_wait_until(0.3 * main_reps)
            finale_ctx.__enter__()
            pT = [finp.tile([P, out_rows], bf16, tag=f"pT{k}", name=f"pT{k}")
                  for k in range(k_tiles)]
            out_sb = [finp.tile([P, S], f32, tag=f"os{m}", name=f"os{m}")
                      for m in range(blk_tiles)]
            for m in range(blk_tiles):
                blk = finp.tile([P, D], f32, tag=f"blk{m}", name=f"blk{m}")
                nc.sync.dma_start(blk[:], cc_out_h[m].ap())
                rs_inv = finp.tile([P, 1], f32, tag=f"ri{m}", name=f"ri{m}")
                nc.scalar.activation(
                    sq_scr[:],
                    blk[:],
                    mybir.ActivationFunctionType.Square,
                    accum_out=rs_inv[:],
                )
                nc.scalar.sqrt(rs_inv[:], rs_inv[:])
                nc.vector.tensor_scalar_max(rs_inv[:], rs_inv[:], 1e-12)
                nc.vector.reciprocal(rs_inv[:], rs_inv[:])
                nc.vector.tensor_tensor(
                    out=rs_inv[:], in0=rs_inv[:], in1=els[:],
                    op=mybir.AluOpType.mult,
                )
                nc.vector.tensor_scalar(
                    out=blk[:],
                    in0=blk[:],
                    scalar1=rs_inv[:],
                    scalar2=None,
                    op0=mybir.AluOpType.mult,
                )
                for k in range(k_tiles):
                    t_ps = fpp.tile([P, P], f32, tag="tps", bufs=1)
                    nc.tensor.transpose(
                        t_ps[:], blk[:, k * P : (k + 1) * P], ident[:]
                    )
                    nc.vector.tensor_copy(
                        out=pT[k][:, m * P : (m + 1) * P], in_=t_ps[:]
                    )
                for n in range(n_tiles_out):
                    o_ps = fpp.tile([P, n_cols], f32, tag="ops", bufs=2)
                    for k in range(k_tiles):
                        nc.tensor.matmul(
                            o_ps[:],
                            lhsT=pT[k][:, m * P : (m + 1) * P],
                            rhs=cet_sb[k][:, n * n_cols : (n + 1) * n_cols],
                            start=(k == 0),
                            stop=(k == k_tiles - 1),
                        )
                    nc.vector.tensor_copy(
                        out=out_sb[m][:, n * n_cols : (n + 1) * n_cols],
                        in_=o_ps[:],
                    )
                nc.sync.dma_start(
                    out.ap()[m * P : (m + 1) * P, :], out_sb[m][:]
                )
            finale_ctx.__exit__(None, None, None)
    nc.compile()
    return nc


def _batch_counts(counts, tiles_per_chunk, batch_tiles, trim=True,
                  force_full_chunks=0):
    """Per-(core, chunk, batch) valid index counts, clamped to the batch.

    The first `force_full_chunks` chunks gather their full padded width
    (pad idx 0 fetches a real row) so the rotating chunk buffers are fully
    initialized before any trimmed chunk can expose stale SBUF bytes."""
    n_cores, n_chunks = counts.shape
    counts = counts.copy()
    if not trim:
        counts[:] = tiles_per_chunk * P
    counts[:, :force_full_chunks] = tiles_per_chunk * P
    n_batches = (tiles_per_chunk + batch_tiles - 1) // batch_tiles
    out = np.zeros((n_cores, n_chunks * n_batches), np.int32)
    for b in range(n_batches):
        start = b * batch_tiles * P
        width_tiles = min(batch_tiles, tiles_per_chunk - b * batch_tiles)
        cap = width_tiles * P
        vals = np.clip(counts - start, 0, cap)
        out[:, b::n_batches] = vals
    return out


def _make_in_maps(adapter_feats, caption_embed, logit_scale, idx16, layers,
                  n_cores, vox_per_core, counts=None, tiles_per_chunk=None,
                  batch_tiles=8, trim=True, force_full_chunks=0):
    import ml_dtypes

    bf = ml_dtypes.bfloat16
    af32 = np.asarray(adapter_feats, np.float32)
    af = np.ascontiguousarray(af32.astype(bf))  # [V, D] bf16
    ce_f32 = np.asarray(caption_embed, np.float32)
    ce_n = ce_f32 / np.clip(
        np.linalg.norm(ce_f32, axis=-1, keepdims=True), 1e-12, None
    )
    cet_np = np.ascontiguousarray(ce_n.T.astype(bf))
    ls = np.asarray(logit_scale, np.float32).reshape(-1)[0]
    ls_rep = np.full((P, 1), ls, np.float32)
    iota_mat = np.ascontiguousarray(
        np.broadcast_to(np.arange(P, dtype=np.float32), (P, P)).astype(bf)
    )
    iotax_mat = np.ascontiguousarray(
        np.broadcast_to(
            np.repeat(np.arange(P, dtype=np.float32), batch_tiles), (P, P * batch_tiles)
        ).astype(bf)
    )
    bc = _batch_counts(np.asarray(counts), tiles_per_chunk, batch_tiles,
                       trim=trim, force_full_chunks=force_full_chunks)
    in_maps = []
    for m in range(n_cores):
        in_maps.append(
            {
                "adapter": af[m * vox_per_core : (m + 1) * vox_per_core],
                "idx16": idx16[m],
                "segf": np.ascontiguousarray(
                    np.concatenate([sa for sa, _ in layers[m]], axis=1)),
                "cntf": np.ascontiguousarray(
                    np.concatenate([ca for _, ca in layers[m]], axis=1)),
                "segfb": np.ascontiguousarray(
                    np.concatenate([sa for sa, _ in layers[m]], axis=1).astype(bf)),
                "iota": iota_mat,
                "iotax": iotax_mat,
                "cet": cet_np,
                "lsr": ls_rep,
                "cnts": bc[m : m + 1],
            }
        )
    return in_maps


def _run(inputs_dict, n_cores, vox_per_core, D, S, batch_tiles=8, trace=False):
    from concourse.bass_utils import run_bass_kernel_spmd

    trim = True
    idx16, layers, tiles_per_chunk, layer_tiles, cnt_tiles, counts = _preprocess(
        inputs_dict["v2p_map"],
        inputs_dict["point_idx"],
        inputs_dict["seg_ids"],
        n_cores,
        vox_per_core,
        S // P,
        trim=True,
    )
    # a zero-valid-count gather would emit no descriptors and never fire its
    # completion semaphore; fall back to untrimmed padding in that case
    if _batch_counts(counts, tiles_per_chunk, batch_tiles, trim=True).min() == 0:
        trim = False
        idx16, layers, tiles_per_chunk, layer_tiles, cnt_tiles, counts = _preprocess(
            inputs_dict["v2p_map"],
            inputs_dict["point_idx"],
            inputs_dict["seg_ids"],
            n_cores,
            vox_per_core,
            S // P,
            trim=False,
        )
    # The first gp_bufs chunks gather untrimmed (pad idx 0 fetches row 0),
    # so every rotating chunk buffer is fully written with finite data
    # before any trimmed chunk can expose stale SBUF bytes.
    import os as _os
    gp_bufs = int(_os.environ.get("GP_BUFS", "13"))
    # stale-SBUF guard: the rotating gather buffers are only safe without
    # an init memset if the first gp_bufs batches are full everywhere
    need_memset = bool(trim) and counts.min() < gp_bufs * batch_tiles * P
    bc = _batch_counts(counts, tiles_per_chunk, batch_tiles, trim=trim)
    full_batches = (bc.min(axis=0) == _batch_counts(
        np.full_like(counts, tiles_per_chunk * P), tiles_per_chunk,
        batch_tiles).min(axis=0)).tolist()
    if _os.environ.get("NO_SHARED_REG"):
        full_batches = [False] * len(full_batches)
    nc = _build_nc(tiles_per_chunk, vox_per_core, D, S, n_cores, layer_tiles,
                   cnt_tiles=cnt_tiles, batch_tiles=batch_tiles,
                   gp_bufs=gp_bufs, full_batches=full_batches,
                   need_memset=need_memset)
    in_maps = _make_in_maps(
        inputs_dict["adapter_feats"],
        inputs_dict["caption_embed"],
        inputs_dict["logit_scale"],
        idx16,
        layers,
        n_cores,
        vox_per_core,
        counts=counts,
        tiles_per_chunk=tiles_per_chunk,
        batch_tiles=batch_tiles,
        trim=trim,
    )
    res = run_bass_kernel_spmd(
        nc, in_maps, core_ids=list(range(n_cores)), trace=trace
    )
    blocks = [res.results[m]["logits_block"] for m in range(n_cores)]
    return _assemble(blocks, S, n_cores), res


def _assemble(blocks, S, n_cores):
    """Core r's output block holds segment rows for chunk r (tile 0) and
    chunk n_cores+r (tile 1)."""
    half = S // 2
    full = np.empty((S, blocks[0].shape[1]), blocks[0].dtype)
    for r in range(n_cores):
        full[r * P : (r + 1) * P] = blocks[r][:P]
        full[half + r * P : half + (r + 1) * P] = blocks[r][P : 2 * P]
    return full


def kernel(adapter_feats, caption_embed, logit_scale, v2p_map, point_idx,
           seg_ids, num_segments=S_FULL, **_):
    logits, _res = _run(
        {
            "adapter_feats": adapter_feats,
            "caption_embed": caption_embed,
            "logit_scale": logit_scale,
            "v2p_map": v2p_map,
            "point_idx": point_idx,
            "seg_ids": seg_ids,
        },
        N_CORES,
        N_VOX // N_CORES,
        D_FULL,
        S_FULL,
    )
    return logits



# revision 32
# speedup vs baseline: 2.0085x; 2.0085x over previous
"""Trainium2 Bass kernel for nn_CaptionHead (segment_reduce).

Computes, for full-size inputs:
    point_feats = adapter_feats[v2p_map]            # [N_PTS, D]
    gathered    = point_feats[point_idx]            # [T, D]
    sums        = segment_sum(gathered, seg_ids, S) # [S, D]
    pooled      = l2norm(sums / max(counts, 1))     # == l2norm(sums)
    logits      = (pooled @ l2norm(ce).T) * exp(logit_scale)

Distribution: adapter_feats is sharded by voxel across the 8 cores
(25000 rows each); host preprocessing composes cidx = v2p_map[point_idx],
routes each point to the core owning its voxel, and deduplicates per
(core, 128-seg chunk, voxel) so each distinct row is fetched once per
chunk with dma_gather (descriptor-count-bound on HW at ~2 ns/row over
4 SWDGE queues).

v2 (427us -> ~250us): scatter weights are NOT built on-device.  The DVE
one-hot builds of v1 ran in 2-port perf mode, which holds the SBUF
shared port pair and locks GpSimd out of writing SWDGE descriptors --
the gather stream stalled behind every build and the main loop degraded
to the SUM of gather+PE+DVE times (427us vs 211 gather / 112 PE alone).
Instead the host pre-expands, per (chunk, 128-row tile), the full
[row, seg] scatter matrix with multiplicities folded in (one tile per
gathered tile, layers merged), stores it as float8_e3m4 (counts <= 15
exact), and the kernel streams it from HBM on the ACT HWDGE queue -- no
Q7 involvement, no DVE.  PSUM accumulation per chunk is evacuated on the
ACT engine too, so the DVE is completely idle during the main loop.

v3 (~250us -> ~220-250ns total @ ~140-170us main loop): two further
descriptor-side wins.  (1) adapter rows are stored float8_e3m4: 256 B
descriptors halve the random-read HBM bytes (quantization RMS ~1.5%
averages out over ~460-row segment means; measured rel err 1.23e-2 vs
the 2e-2 gate).  (2) single_packet=0 packs multiple gather descriptors
per SDMA packet (nomm slope 159 -> 118us).  (3) four DENSE chunks skip
the gather entirely: the full fp8 shard (25088x256, 50 KB/partition) is
DMA'd contiguously into SBUF once at the prologue, and those chunks'
scatter weights cover all 196 shard tiles (row slot == shard-local voxel
index) so their matmuls read SBUF directly -- zero descriptors, at the
cost of 196-52 extra matmul tiles per dense chunk.  Four dense chunks
balance the PE (~107ns/tile, ~1400 tiles total) against the Q7/SDMA
descriptor stream of the twelve gathered chunks under the observed
machine-load drift; keeping them off chunks 7/15 keeps the half-boundary
ReduceScatter critical path short (tlsim prologue 94 -> 84us).

The [S, D] partial sums ReduceScatter (f32 -- bf16 collectives hang) in
two halves overlapped with the loop; each core normalizes its 256
segment rows (1/count cancels in the l2norm) and multiplies against
host-prenormalized caption embeddings (bf16).  Core r returns logits
rows for chunks r and 8+r; the host concatenates.
"""

import math
import os

import numpy as np

N_VOX = 200000
N_PTS = 500000
T_FULL = 1000000
S_FULL = 2048
D_FULL = 256
N_CORES = 8
P = 128
# chunks computed from the SBUF-resident shard instead of gathering
# (comma-separated chunk ids; overridable via env DENSE_CHUNKS).  Four dense
# chunks balance the PE cost (196 extra matmul tiles each) against the
# per-row gather descriptor cost of the other twelve; keeping them off the
# half boundaries (7, 15) keeps the ReduceScatter critical path short.
DEFAULT_DENSE_CHUNKS = "3,6,10,13"


def _preprocess(v2p_map, point_idx, seg_ids, n_cores, vox_per_core, n_chunks,
                trim=True, dense_chunks=()):
    """Route points to voxel-owning cores, dedup per (core, chunk, voxel).

    Chunks in dense_chunks are not gathered at all: their scatter weights are
    expanded over the core's FULL voxel shard (row slot == shard-local voxel
    index) so the matmuls read the contiguously-preloaded shard from SBUF.

    Returns (idx16, oh, oh_d, tiles_per_chunk, counts, oh_is_fp8):
      idx16[m]: [128, NIDX//16] int16 shard-local voxel index per row in
                dma_gather's 16-partition-wrapped, 8x-replicated layout
                (gathered chunks only, packed by gather slot).
      oh[m]:    [128, n_gather*tiles_per_chunk*128] scatter weights; column
                block (gi*tpc + t)*128 + s holds, at partition r, the
                multiplicity of (segment s, row t*128+r) in gather slot gi
                (0 if absent; padding rows are all-zero so gathered garbage
                in trimmed slots cannot leak).
      oh_d[m]:  [128, n_dense*ntile_d*128] same, per dense slot, with the
                row slot equal to the shard-local voxel index.
      counts:   [n_cores, n_gather] valid row count per gathered cell.
    """
    import ml_dtypes

    dense_chunks = sorted(dense_chunks)
    gathered = [c for c in range(n_chunks) if c not in dense_chunks]
    g_slot = np.full(n_chunks, -1, np.int64)
    for i, c in enumerate(gathered):
        g_slot[c] = i
    d_slot = np.full(n_chunks, -1, np.int64)
    for j, c in enumerate(dense_chunks):
        d_slot[c] = j
    n_gather = len(gathered)
    n_dense = len(dense_chunks)
    ntile_d = -(-vox_per_core // P)

    v2p = np.asarray(v2p_map).astype(np.int64)
    pidx = np.asarray(point_idx).astype(np.int64)
    seg = np.asarray(seg_ids).astype(np.int64)
    cidx = v2p[pidx]                      # composed voxel index per point
    core = cidx // vox_per_core
    lvox = cidx - core * vox_per_core
    chunk = seg >> 7                      # 128 segments per chunk
    segl = seg & 127
    cell = core * n_chunks + chunk        # [0, n_cores*n_chunks)
    key3 = (cell * vox_per_core + lvox) * 128 + segl
    uk, ucnt = np.unique(key3, return_counts=True)      # sorted pairs
    # fp8_e3m4 is exact for integers <= 15; overflow to bf16 in the
    # astronomically-unlikely case
    oh_is_fp8 = ucnt.max() <= 15
    oh_np_dt = ml_dtypes.float8_e3m4 if oh_is_fp8 else ml_dtypes.bfloat16
    pair_cell = uk // (vox_per_core * 128)
    pair_chunk = pair_cell % n_chunks
    pair_core = pair_cell // n_chunks
    pair_lvox = (uk // 128) % vox_per_core
    useg = (uk % 128).astype(np.int64)
    is_dense = d_slot[pair_chunk] >= 0

    # --- dense chunks: row slot == shard-local voxel index ---
    oh_d = None
    if n_dense:
        npd = ntile_d * P
        ohd_arr = np.zeros((n_cores * n_dense, npd, P), oh_np_dt)
        dm = is_dense
        dcell = pair_core[dm] * n_dense + d_slot[pair_chunk[dm]]
        ohd_arr[dcell, pair_lvox[dm], useg[dm]] = ucnt[dm].astype(np.float32)
        oh_d = []
        for m in range(n_cores):
            o = ohd_arr.reshape(n_cores, n_dense, ntile_d, P, P)[m]
            oh_d.append(np.ascontiguousarray(
                o.transpose(2, 0, 1, 3).reshape(P, n_dense * npd)))

    # --- gathered chunks: dedup rows, pack by in-cell slot ---
    gm = ~is_dense
    # gcell in [0, n_cores*n_gather) by gather slot
    gcell = pair_core[gm] * n_gather + g_slot[pair_chunk[gm]]
    rowkey = gcell * vox_per_core + pair_lvox[gm]
    urow, row_inv = np.unique(rowkey, return_inverse=True)
    rcell = (urow // vox_per_core).astype(np.int64)
    rvox = (urow % vox_per_core).astype(np.int64)
    counts = np.bincount(rcell, minlength=n_cores * n_gather)
    tiles_per_chunk = max(1, math.ceil(counts.max() / P))
    npc = tiles_per_chunk * P
    offs = np.concatenate([[0], np.cumsum(counts)])
    pos = np.arange(len(urow)) - offs[rcell]            # in-cell row slot
    vox_arr = np.full((n_cores * n_gather, npc), -1 if trim else 0, np.int16)
    vox_arr[rcell, pos] = rvox.astype(np.int16)
    oh = np.zeros((n_cores * n_gather, npc, P), oh_np_dt)
    pr_cell = rcell[row_inv]
    pr_pos = pos[row_inv]
    oh[pr_cell, pr_pos, useg[gm]] = ucnt[gm].astype(np.float32)
    idx16 = []
    ohs = []
    for m in range(n_cores):
        arr = vox_arr.reshape(n_cores, n_gather, npc)[m].reshape(-1, 16).T
        idx16.append(np.ascontiguousarray(np.tile(arr, (8, 1))))
        o = oh.reshape(n_cores, n_gather, tiles_per_chunk, P, P)[m]
        # [slot, tile, r, seg] -> [r, slot, tile, seg] -> [128, NT*128]
        ohs.append(np.ascontiguousarray(
            o.transpose(2, 0, 1, 3).reshape(P, n_gather * npc)))
    return idx16, ohs, oh_d, tiles_per_chunk, \
        counts.reshape(n_cores, n_gather), oh_is_fp8


def _batch_counts(counts, tiles_per_chunk, batch_tiles, trim=True,
                  force_full_chunks=0):
    """Per-(core, chunk, batch) valid index counts, clamped to the batch."""
    n_cores, n_chunks = counts.shape
    counts = counts.copy()
    if not trim:
        counts[:] = tiles_per_chunk * P
    counts[:, :force_full_chunks] = tiles_per_chunk * P
    n_batches = (tiles_per_chunk + batch_tiles - 1) // batch_tiles
    out = np.zeros((n_cores, n_chunks * n_batches), np.int32)
    for b in range(n_batches):
        start = b * batch_tiles * P
        width_tiles = min(batch_tiles, tiles_per_chunk - b * batch_tiles)
        cap = width_tiles * P
        vals = np.clip(counts - start, 0, cap)
        out[:, b::n_batches] = vals
    return out


def _build_nc(tiles_per_chunk, vox_per_core, D, S, n_cores, batch_tiles=8,
              main_reps=1, mode="full", single_core=False, gp_bufs=13,
              oh_bufs=3, acc_bufs=4, full_batches=None, need_memset=False,
              gather_dt="bf16", oh_is_fp8=True, dense_chunks=()):
    """mode: "full" | "nomm" (gathers only) | "nogather" (oh stream + matmul
    on constant data) | "noonehot" (gather + matmul, constant weights).
    main_reps repeats the main loop; with mode="full" the output stays
    correct (each rep recomputes the same sums; only the last is copied)."""
    import concourse.bacc as bacc
    import concourse.mybir as mybir
    import concourse.tile as tile
    from concourse.masks import make_identity

    f32 = mybir.dt.float32
    bf16 = mybir.dt.bfloat16
    f8e3 = mybir.dt.float8e3
    i16 = mybir.dt.int16
    i32 = mybir.dt.int32
    gdt = bf16 if gather_dt == "bf16" else f8e3
    odt = f8e3 if oh_is_fp8 else bf16
    n_chunks = S // P
    dense_chunks = sorted(dense_chunks)
    gathered = [c for c in range(n_chunks) if c not in dense_chunks]
    g_slot = {c: i for i, c in enumerate(gathered)}
    d_slot = {c: j for j, c in enumerate(dense_chunks)}
    n_gather = len(gathered)
    n_dense = len(dense_chunks)
    ntile_d = -(-vox_per_core // P)
    npd = ntile_d * P
    NT = n_gather * tiles_per_chunk            # total gathered point tiles
    NIDX = NT * P                              # total gathered rows
    npc = tiles_per_chunk * P
    # dense scatter-weight tables stream in pieces sized to the rotating
    # oh buffer (whose width serves both chunk kinds)
    oh_cols = max(npc, npd // 4 if n_dense else 0)
    piece_tiles = oh_cols // P
    if n_dense:
        # dense scatter-weight pieces rotate 4x faster than per-chunk tables
        oh_bufs = max(oh_bufs, 6)
    # multi-descriptor packets cut the per-row SDMA packet overhead of the
    # gather stream by ~25% (nomm slope 159us -> 118us)
    single_packet = os.environ.get("SINGLE_PACKET", "0") == "1"
    out_rows = S // n_cores                    # 256
    blk_tiles = out_rows // P                  # 2
    k_tiles = D // P                           # 2
    n_cols = 512                               # moving-operand tile width
    n_tiles_out = S // n_cols                  # 4

    nc = bacc.Bacc(
        "TRN2",
        target_bir_lowering=False,
        debug=False,
        enable_asserts=False,
        num_devices=n_cores,
        # SWDGE descriptor-ring carveout: must hold two in-flight
        # dma_gathers of batch_tiles*128 descriptors each.
        dynamic_dma_scratch_size=32768,
        # round-robin gathers over all 4 SWDGE queues: each queue's
        # descriptor generation runs on its own Q7 core pair.
        num_swdge_queues=4,
    )

    adapter = nc.dram_tensor("adapter", [vox_per_core, D], gdt,
                             kind="ExternalInput")
    idx16 = nc.dram_tensor("idx16", [P, NIDX // 16], i16, kind="ExternalInput")
    # host-pre-expanded scatter weights: per gather slot gi, tile t the lhsT
    # block [128 rows, 128 segs] lives at columns (gi*tpc + t)*128.
    ohd = nc.dram_tensor("ohd", [P, NT * P], odt, kind="ExternalInput")
    ohd_d = adapter_w = None
    if n_dense:
        ohd_d = nc.dram_tensor("ohd_d", [P, n_dense * npd], odt,
                               kind="ExternalInput")
        # the full shard, 128-partition-wrapped ([p, t, :] = voxel t*128+p),
        # fp8 so it stays SBUF-resident next to the gather buffers
        adapter_w = nc.dram_tensor("adapter_w", [P, ntile_d * D], f8e3,
                                   kind="ExternalInput")
    # caption embeds arrive L2-normalized from the host; only the transposed
    # copy is needed for the logits matmul.
    cet = nc.dram_tensor("cet", [D, S], bf16, kind="ExternalInput")
    lsr = nc.dram_tensor("lsr", [P, 1], f32, kind="ExternalInput")
    n_batches = (tiles_per_chunk + batch_tiles - 1) // batch_tiles
    if full_batches is None:
        full_batches = [False] * (n_gather * n_batches)
    cnts = nc.dram_tensor("cnts", [1, n_gather * n_batches], i32,
                          kind="ExternalInput")
    out = nc.dram_tensor("logits_block", [out_rows, S], f32,
                         kind="ExternalOutput")
    cc_in = nc.dram_tensor("cc_in", [S, D], f32, kind="Internal")
    half_rows = S // 2
    cc_out_h = [
        nc.dram_tensor(f"cc_out{h}", [half_rows // n_cores, D], f32,
                       kind="Internal")
        for h in range(2)
    ]

    with tile.TileContext(nc) as tc:
        with (
            tc.tile_pool(name="const", bufs=1) as constp,
            tc.tile_pool(name="gather", bufs=gp_bufs) as gp,
            tc.tile_pool(name="oh", bufs=oh_bufs) as ohp,
            tc.tile_pool(name="misc", bufs=1) as miscp,
            tc.tile_pool(name="fin", bufs=1) as finp,
            tc.tile_pool(name="fpsum", bufs=1, space="PSUM") as fpp,
        ):
            # ---- prologue loads, shortest-critical-path first ----
            # SP (sync) queue: batch counts + the first idx stripe unblock
            # the first gather within ~2 us.
            cnt_sb = constp.tile([1, n_gather * n_batches], i32)
            nc.sync.dma_start(cnt_sb[:], cnts.ap())
            # idx stripes are separate tiles so a gather only depends on the
            # stripe that covers its chunk (dep tracking is per-tile).
            chunk_cols = tiles_per_chunk * P // 16
            stripe_chunks = max(1, 2048 // chunk_cols)
            stripe_cols = stripe_chunks * chunk_cols
            stripe_bounds = []
            for s0 in range(0, NIDX // 16, stripe_cols):
                stripe_bounds.append((s0, min(s0 + stripe_cols, NIDX // 16)))
            idx_parts = [
                constp.tile([P, s1 - s0], i16, name=f"idx{s0}")
                for s0, s1 in stripe_bounds
            ]
            nc.sync.dma_start(idx_parts[0][:], idx16.ap()[:, : stripe_bounds[0][1]])
            ls_sb = finp.tile([P, 1], f32)
            nc.sync.dma_start(ls_sb[:], lsr.ap())
            # Later stripes aren't needed until chunk 4+; hint them behind
            # the first gathers so they don't hog the DMA engines up front.
            with tc.tile_wait_until(0.02):
                for (s0, s1), part in zip(stripe_bounds[1:], idx_parts[1:]):
                    nc.sync.dma_start(part[:], idx16.ap()[:, s0:s1])
            shard_sb = None
            if n_dense:
                # full fp8 shard, loaded contiguously once (no descriptors):
                # feeds every dense chunk's matmuls from SBUF
                shard_sb = miscp.tile([P, ntile_d, D], f8e3, name="shard")
                with tc.tile_wait_until(0.01):
                    nc.sync.dma_start(shard_sb[:], adapter_w.ap())
            els = finp.tile([P, 1], f32)
            nc.scalar.activation(els[:], ls_sb[:], mybir.ActivationFunctionType.Exp)
            ident = constp.tile([P, P], f32)
            make_identity(nc, ident[:])
            ident_w = constp.tile([P, P], odt)
            nc.vector.tensor_copy(out=ident_w[:], in_=ident[:])

            sums_sb = miscp.tile([P, n_chunks * D], f32)
            sq_scr = finp.tile([P, D], f32)

            # ACT queue: the transposed caption embeds (finale-only input).
            cet_sb = [finp.tile([P, S], bf16, tag=f"cet{k}", name=f"cet{k}")
                      for k in range(k_tiles)]
            for k in range(k_tiles):
                nc.scalar.dma_start(cet_sb[k][:], cet.ap()[k * P : (k + 1) * P, :])

            # ---- main: gather + scatter-weight matmul segment reduction ----
            # Chunk-staged pipeline: all of chunk c's rows are gathered into
            # one chunk-wide buffer (2 in flight), the chunk's scatter
            # weights stream in on the ACT HWDGE queue, and its matmuls run
            # as a single burst.
            if need_memset:
                for _slot in range(gp_bufs):
                    g_init = gp.tile([P, batch_tiles, D], gdt, tag="g",
                                     name="g_init")
                    nc.vector.memset(g_init[:], 0)
            g_static = None
            if mode == "nogather":
                g_static = miscp.tile([P, batch_tiles, D], gdt)
                nc.vector.memset(g_static[:], 1.0)
            if mode == "nomm":
                nc.vector.memset(sums_sb[:], 1.0)
            # shared register holding the full batch count: only batches that
            # are partial on some core pay a per-batch reg_load.
            vreg_full = None
            if any(full_batches) and mode != "nogather":
                vreg_full = nc.gpsimd.alloc_register()
                nc.gpsimd.reg_mov(vreg_full, batch_tiles * P)

            with tc.tile_pool(name="acc", bufs=acc_bufs, space="PSUM") as accp:
                for rep in range(main_reps):
                    for c in range(n_chunks):
                        dense = c in d_slot
                        acc = None
                        if mode != "nomm":
                            acc = accp.tile([P, D], f32, tag="acc", name="acc")
                        if dense:
                            if mode == "nomm":
                                continue
                            # dense chunk: matmuls read the SBUF-resident
                            # shard (no gather); scatter weights stream in
                            # oh-buffer-sized pieces
                            dj = d_slot[c]
                            t = 0
                            while t < ntile_d:
                                pt = min(piece_tiles, ntile_d - t)
                                piece = None
                                if mode != "noonehot":
                                    piece = ohp.tile([P, oh_cols], odt,
                                                     tag="oh", name="oh")
                                    col = (dj * ntile_d + t) * P
                                    nc.scalar.dma_start(
                                        piece[:, : pt * P],
                                        ohd_d.ap()[:, col : col + pt * P],
                                    )
                                for j in range(pt):
                                    oh_ap = (ident_w[:, :]
                                             if mode == "noonehot"
                                             else piece[:, j * P : (j + 1) * P])
                                    nc.tensor.matmul(
                                        acc[:],
                                        lhsT=oh_ap,
                                        rhs=shard_sb[:, t + j, :],
                                        start=(t + j == 0),
                                        stop=(t + j == ntile_d - 1),
                                    )
                                t += pt
                        else:
                            gi = g_slot[c]
                            oh_sb = None
                            if mode in ("full", "nogather"):
                                oh_sb = ohp.tile([P, oh_cols], odt,
                                                 tag="oh", name="oh")
                                nc.scalar.dma_start(
                                    oh_sb[:, :npc],
                                    ohd.ap()[:, gi * npc : (gi + 1) * npc],
                                )
                            gtiles = []
                            done = 0
                            while done < tiles_per_chunk:
                                bt = min(batch_tiles, tiles_per_chunk - done)
                                if mode == "nogather":
                                    done += bt
                                    continue
                                g = gp.tile([P, batch_tiles, D], gdt,
                                            tag="g", name="g")
                                gtiles.append(g)
                                col0 = (gi * tiles_per_chunk + done) * P // 16
                                nidx = bt * P
                                bidx = gi * n_batches + done // batch_tiles
                                part = idx_parts[col0 // stripe_cols]
                                pc0 = col0 % stripe_cols
                                if full_batches[bidx] and bt == batch_tiles:
                                    vreg = vreg_full
                                else:
                                    vreg = nc.gpsimd.alloc_register()
                                    nc.gpsimd.reg_load(
                                        vreg, cnt_sb[0:1, bidx : bidx + 1]
                                    )
                                nc.gpsimd.dma_gather(
                                    out_ap=g[:, :bt, :],
                                    in_ap=adapter.ap(),
                                    idxs_ap=part[:, pc0 : pc0 + nidx // 16],
                                    num_idxs=nidx,
                                    num_idxs_reg=vreg,
                                    elem_size=D,
                                    queue_num=bidx % 4,
                                    single_packet=single_packet,
                                )
                                if vreg is not vreg_full:
                                    nc.gpsimd.free_register(vreg)
                                done += bt
                            if mode == "nomm":
                                continue
                            for t in range(tiles_per_chunk):
                                if mode == "noonehot":
                                    oh_ap = ident_w[:, :]
                                else:
                                    oh_ap = oh_sb[:, t * P : (t + 1) * P]
                                rhs = (g_static[:, 0, :]
                                       if mode == "nogather"
                                       else gtiles[t // batch_tiles]
                                       [:, t % batch_tiles, :])
                                nc.tensor.matmul(
                                    acc[:],
                                    lhsT=oh_ap,
                                    rhs=rhs,
                                    start=(t == 0),
                                    stop=(t == tiles_per_chunk - 1),
                                )
                        if rep == main_reps - 1:
                            # evacuate PSUM on the ACT engine (its own SBUF
                            # port; keeps DVE idle so GpSimd never loses the
                            # shared port pair mid-gather-stream)
                            nc.scalar.activation(
                                sums_sb[:, c * D : (c + 1) * D],
                                acc[:],
                                mybir.ActivationFunctionType.Copy,
                            )
                            # stage this chunk's partial sums (ACT HWDGE queue
                            # so the SP queue stays free for other loads)
                            nc.scalar.dma_start(
                                cc_in.ap()[c * P : (c + 1) * P, :],
                                sums_sb[:, c * D : (c + 1) * D],
                            )
                            if c in (n_chunks // 2 - 1, n_chunks - 1):
                                h = 0 if c < n_chunks // 2 else 1
                                lo = h * half_rows
                                if single_core:
                                    nc.sync.dma_start(
                                        cc_out_h[h].ap(),
                                        cc_in.ap()[lo : lo + P, :],
                                    )
                                else:
                                    nc.gpsimd.collective_compute(
                                        "ReduceScatter",
                                        mybir.AluOpType.add,
                                        replica_groups=[list(range(n_cores))],
                                        ins=[cc_in.ap()[lo : lo + half_rows, :]],
                                        outs=[cc_out_h[h].ap()],
                                    )

            # ---- finale: per half-block normalize + logits rows ----
            # Pin the finale to the end of the schedule: without this the
            # tile scheduler hoists it into the middle of the main loop
            # (its collective input *can* be ready early), where it
            # head-of-line blocks the PE/DVE queues and stalls the gather
            # buffer recycling.
            finale_ctx = tc.tile_wait_until(0.3 * main_reps)
            finale_ctx.__enter__()
            pT = [finp.tile([P, out_rows], bf16, tag=f"pT{k}", name=f"pT{k}")
                  for k in range(k_tiles)]
            out_sb = [finp.tile([P, S], f32, tag=f"os{m}", name=f"os{m}")
                      for m in range(blk_tiles)]
            for m in range(blk_tiles):
                blk = finp.tile([P, D], f32, tag=f"blk{m}", name=f"blk{m}")
                nc.sync.dma_start(blk[:], cc_out_h[m].ap())
                rs_inv = finp.tile([P, 1], f32, tag=f"ri{m}", name=f"ri{m}")
                nc.scalar.activation(
                    sq_scr[:],
                    blk[:],
                    mybir.ActivationFunctionType.Square,
                    accum_out=rs_inv[:],
                )
                nc.scalar.sqrt(rs_inv[:], rs_inv[:])
                nc.vector.tensor_scalar_max(rs_inv[:], rs_inv[:], 1e-12)
                nc.vector.reciprocal(rs_inv[:], rs_inv[:])
                nc.vector.tensor_tensor(
                    out=rs_inv[:], in0=rs_inv[:], in1=els[:],
                    op=mybir.AluOpType.mult,
                )
                nc.vector.tensor_scalar(
                    out=blk[:],
                    in0=blk[:],
                    scalar1=rs_inv[:],
                    scalar2=None,
                    op0=mybir.AluOpType.mult,
                )
                for k in range(k_tiles):
                    t_ps = fpp.tile([P, P], f32, tag="tps", bufs=1)
                    nc.tensor.transpose(
                        t_ps[:], blk[:, k * P : (k + 1) * P], ident[:]
                    )
                    nc.vector.tensor_copy(
                        out=pT[k][:, m * P : (m + 1) * P], in_=t_ps[:]
                    )
                for n in range(n_tiles_out):
                    o_ps = fpp.tile([P, n_cols], f32, tag="ops", bufs=2)
                    for k in range(k_tiles):
                        nc.tensor.matmul(
                            o_ps[:],
                            lhsT=pT[k][:, m * P : (m + 1) * P],
                            rhs=cet_sb[k][:, n * n_cols : (n + 1) * n_cols],
                            start=(k == 0),
                            stop=(k == k_tiles - 1),
                        )
                    nc.vector.tensor_copy(
                        out=out_sb[m][:, n * n_cols : (n + 1) * n_cols],
                        in_=o_ps[:],
                    )
                nc.sync.dma_start(
                    out.ap()[m * P : (m + 1) * P, :], out_sb[m][:]
                )
            finale_ctx.__exit__(None, None, None)
    nc.compile()
    return nc


def _prep(inputs_dict, n_cores, vox_per_core, D, S, batch_tiles=8):
    """Host preprocessing + derived build parameters, shared by kernel()
    and the benches."""
    gp_bufs = int(os.environ.get("GP_BUFS", "13"))
    oh_bufs = int(os.environ.get("OH_BUFS", "3"))
    dc_env = os.environ.get("DENSE_CHUNKS", DEFAULT_DENSE_CHUNKS)
    dense_chunks = tuple(
        int(x) for x in dc_env.split(",") if x.strip() != "")
    trim = True
    idx16, ohs, oh_d, tiles_per_chunk, counts, oh_is_fp8 = _preprocess(
        inputs_dict["v2p_map"], inputs_dict["point_idx"],
        inputs_dict["seg_ids"], n_cores, vox_per_core, S // P, trim=True,
        dense_chunks=dense_chunks,
    )
    # The first chunks covering all gp_bufs rotating buffers gather
    # untrimmed (pad idx 0 fetches a real row; its scatter weight is 0) so
    # every buffer is fully written with finite data before any trimmed
    # chunk can expose stale SBUF bytes.  Patch the pad slots of those
    # chunks from -1 to 0 in the wrapped idx16 layout.
    n_batches = -(-tiles_per_chunk // batch_tiles)
    ffc = min(-(-gp_bufs // n_batches) + 1, S // P)
    npc16 = tiles_per_chunk * P // 16
    for m in range(n_cores):
        head = idx16[m][:, : ffc * npc16]
        head[head < 0] = 0
    # a zero-valid-count gather would emit no descriptors and never fire its
    # completion semaphore; fall back to untrimmed padding in that case
    if _batch_counts(counts, tiles_per_chunk, batch_tiles, trim=True,
                     force_full_chunks=ffc).min() == 0:
        trim = False
        ffc = 0
        idx16, ohs, oh_d, tiles_per_chunk, counts, oh_is_fp8 = _preprocess(
            inputs_dict["v2p_map"], inputs_dict["point_idx"],
            inputs_dict["seg_ids"], n_cores, vox_per_core, S // P, trim=False,
            dense_chunks=dense_chunks,
        )
    need_memset = False
    bc = _batch_counts(counts, tiles_per_chunk, batch_tiles, trim=trim,
                       force_full_chunks=ffc)
    full_batches = (bc.min(axis=0) == _batch_counts(
        np.full_like(counts, tiles_per_chunk * P), tiles_per_chunk,
        batch_tiles).min(axis=0)).tolist()
    # fp8_e3m4 rows: 256 B descriptors, half the random-read HBM bytes of
    # bf16; quantization (RMS ~1.5%) averages out over ~460-row segment
    # means and lands well under the 2e-2 gate (measured 1.23e-2)
    gather_dt = os.environ.get("GATHER_DT", "fp8")
    return dict(
        idx16=idx16, ohs=ohs, oh_d=oh_d, dense_chunks=dense_chunks,
        tiles_per_chunk=tiles_per_chunk, counts=counts,
        oh_is_fp8=oh_is_fp8, trim=trim, need_memset=need_memset, ffc=ffc,
        full_batches=full_batches, gp_bufs=gp_bufs, oh_bufs=oh_bufs,
        batch_tiles=batch_tiles,
        gather_dt=gather_dt, n_cores=n_cores, vox_per_core=vox_per_core,
        D=D, S=S,
    )


def _nc_from_prep(prep, main_reps=1, mode="full"):
    return _build_nc(
        prep["tiles_per_chunk"], prep["vox_per_core"], prep["D"], prep["S"],
        prep["n_cores"], batch_tiles=prep["batch_tiles"],
        main_reps=main_reps, mode=mode, gp_bufs=prep["gp_bufs"],
        oh_bufs=prep["oh_bufs"],
        full_batches=prep["full_batches"], need_memset=prep["need_memset"],
        gather_dt=prep["gather_dt"], oh_is_fp8=prep["oh_is_fp8"],
        dense_chunks=prep["dense_chunks"],
        single_core=os.environ.get("SINGLE_CORE", "0") == "1",
    )


def _make_in_maps(prep, inputs_dict):
    import ml_dtypes

    bf = ml_dtypes.bfloat16
    gather_np_dt = bf if prep["gather_dt"] == "bf16" else ml_dtypes.float8_e3m4
    n_cores = prep["n_cores"]
    vox_per_core = prep["vox_per_core"]
    af32 = np.asarray(inputs_dict["adapter_feats"], np.float32)
    af = np.ascontiguousarray(af32.astype(gather_np_dt))
    ce_f32 = np.asarray(inputs_dict["caption_embed"], np.float32)
    ce_n = ce_f32 / np.clip(
        np.linalg.norm(ce_f32, axis=-1, keepdims=True), 1e-12, None
    )
    cet_np = np.ascontiguousarray(ce_n.T.astype(bf))
    ls = np.asarray(inputs_dict["logit_scale"], np.float32).reshape(-1)[0]
    ls_rep = np.full((P, 1), ls, np.float32)
    bc = _batch_counts(prep["counts"], prep["tiles_per_chunk"],
                       prep["batch_tiles"], trim=prep["trim"],
                       force_full_chunks=prep["ffc"])
    n_dense = len(prep["dense_chunks"])
    if n_dense:
        ntile_d = -(-vox_per_core // P)
        af8 = af32.astype(ml_dtypes.float8_e3m4)
        af8_pad = np.zeros((n_cores, ntile_d * P, D_FULL), af8.dtype)
        af8_pad[:, : vox_per_core] = af8.reshape(n_cores, vox_per_core, -1)
        # 128-partition-wrapped: [p, t*D + d] = voxel t*128+p
        af_w = np.ascontiguousarray(
            af8_pad.reshape(n_cores, ntile_d, P, D_FULL)
            .transpose(0, 2, 1, 3).reshape(n_cores, P, ntile_d * D_FULL))
    in_maps = []
    for m in range(n_cores):
        im = {
            "adapter": af[m * vox_per_core : (m + 1) * vox_per_core],
            "idx16": prep["idx16"][m],
            "ohd": prep["ohs"][m],
            "cet": cet_np,
            "lsr": ls_rep,
            "cnts": bc[m : m + 1],
        }
        if n_dense:
            im["ohd_d"] = prep["oh_d"][m]
            im["adapter_w"] = af_w[m]
        in_maps.append(im)
    return in_maps


def _run(inputs_dict, n_cores, vox_per_core, D, S, batch_tiles=8, trace=False):
    from concourse.bass_utils import run_bass_kernel_spmd

    prep = _prep(inputs_dict, n_cores, vox_per_core, D, S,
                 batch_tiles=batch_tiles)
    nc = _nc_from_prep(prep, main_reps=1, mode="full")
    in_maps = _make_in_maps(prep, inputs_dict)
    res = run_bass_kernel_spmd(
        nc, in_maps, core_ids=list(range(n_cores)), trace=trace
    )
    blocks = [res.results[m]["logits_block"] for m in range(n_cores)]
    return _assemble(blocks, S, n_cores), res


def _assemble(blocks, S, n_cores):
    """Core r's output block holds segment rows for chunk r (tile 0) and
    chunk n_cores+r (tile 1)."""
    half = S // 2
    full = np.empty((S, blocks[0].shape[1]), blocks[0].dtype)
    for r in range(n_cores):
        full[r * P : (r + 1) * P] = blocks[r][:P]
        full[half + r * P : half + (r + 1) * P] = blocks[r][P : 2 * P]
    return full


def kernel(adapter_feats, caption_embed, logit_scale, v2p_map, point_idx,
           seg_ids, num_segments=S_FULL, **_):
    logits, _res = _run(
        {
            "adapter_feats": adapter_feats,
            "caption_embed": caption_embed,
            "logit_scale": logit_scale,
            "v2p_map": v2p_map,
            "point_idx": point_idx,
            "seg_ids": seg_ids,
        },
        N_CORES,
        N_VOX // N_CORES,
        D_FULL,
        S_FULL,
    )
    return logits


# revision 35
# speedup vs baseline: 2.3122x; 1.1512x over previous
"""Trainium2 Bass kernel for nn_CaptionHead (segment_reduce).

Computes, for full-size inputs:
    point_feats = adapter_feats[v2p_map]            # [N_PTS, D]
    gathered    = point_feats[point_idx]            # [T, D]
    sums        = segment_sum(gathered, seg_ids, S) # [S, D]
    pooled      = l2norm(sums / max(counts, 1))     # == l2norm(sums)
    logits      = (pooled @ l2norm(ce).T) * exp(logit_scale)

Distribution: adapter_feats is sharded by voxel across the 8 cores
(25000 rows each); host preprocessing composes cidx = v2p_map[point_idx],
routes each point to the core owning its voxel, and deduplicates per
(core, 128-seg chunk, voxel) so each distinct row is fetched once per
chunk with dma_gather (descriptor-count-bound on HW at ~2 ns/row over
4 SWDGE queues).

v2 (427us -> ~250us): scatter weights are NOT built on-device.  The DVE
one-hot builds of v1 ran in 2-port perf mode, which holds the SBUF
shared port pair and locks GpSimd out of writing SWDGE descriptors --
the gather stream stalled behind every build and the main loop degraded
to the SUM of gather+PE+DVE times (427us vs 211 gather / 112 PE alone).
Instead the host pre-expands, per (chunk, 128-row tile), the full
[row, seg] scatter matrix with multiplicities folded in (one tile per
gathered tile, layers merged), stores it as float8_e3m4 (counts <= 15
exact), and the kernel streams it from HBM on the ACT HWDGE queue -- no
Q7 involvement, no DVE.  PSUM accumulation per chunk is evacuated on the
ACT engine too, so the DVE is completely idle during the main loop.

v3 (~250us -> ~220-250ns total @ ~140-170us main loop): two further
descriptor-side wins.  (1) adapter rows are stored float8_e3m4: 256 B
descriptors halve the random-read HBM bytes (quantization RMS ~1.5%
averages out over ~460-row segment means; measured rel err 1.23e-2 vs
the 2e-2 gate).  (2) single_packet=0 packs multiple gather descriptors
per SDMA packet (nomm slope 159 -> 118us).  (3) four DENSE chunks skip
the gather entirely: the full fp8 shard (25088x256, 50 KB/partition) is
DMA'd contiguously into SBUF once at the prologue, and those chunks'
scatter weights cover all 196 shard tiles (row slot == shard-local voxel
index) so their matmuls read SBUF directly -- zero descriptors, at the
cost of 196-52 extra matmul tiles per dense chunk.  Four dense chunks
balance the PE (~107ns/tile, ~1400 tiles total) against the Q7/SDMA
descriptor stream of the twelve gathered chunks under the observed
machine-load drift; keeping them off chunks 7/15 keeps the half-boundary
ReduceScatter critical path short (tlsim prologue 94 -> 84us).

The [S, D] partial sums ReduceScatter (f32 -- bf16 collectives hang) in
two halves overlapped with the loop; each core normalizes its 256
segment rows (1/count cancels in the l2norm) and multiplies against
host-prenormalized caption embeddings (bf16).  Core r returns logits
rows for chunks r and 8+r; the host concatenates.
"""

import math
import os

import numpy as np

N_VOX = 200000
N_PTS = 500000
T_FULL = 1000000
S_FULL = 2048
D_FULL = 256
N_CORES = 8
P = 128
# chunks computed from the SBUF-resident shard instead of gathering
# (comma-separated chunk ids; overridable via env DENSE_CHUNKS).  Four dense
# chunks balance the PE cost (196 extra matmul tiles each) against the
# per-row gather descriptor cost of the other twelve; keeping them off the
# half boundaries (7, 15) keeps the ReduceScatter critical path short.
DEFAULT_DENSE_CHUNKS = "3,6,10,13"


def _preprocess(v2p_map, point_idx, seg_ids, n_cores, vox_per_core, n_chunks,
                trim=True, dense_chunks=()):
    """Route points to voxel-owning cores, dedup per (core, chunk, voxel).

    Chunks in dense_chunks are not gathered at all: their scatter weights are
    expanded over the core's FULL voxel shard (row slot == shard-local voxel
    index) so the matmuls read the contiguously-preloaded shard from SBUF.

    Returns (idx16, oh, oh_d, tiles_per_chunk, counts, oh_is_fp8):
      idx16[m]: [128, NIDX//16] int16 shard-local voxel index per row in
                dma_gather's 16-partition-wrapped, 8x-replicated layout
                (gathered chunks only, packed by gather slot).
      oh[m]:    [128, n_gather*tiles_per_chunk*128] scatter weights; column
                block (gi*tpc + t)*128 + s holds, at partition r, the
                multiplicity of (segment s, row t*128+r) in gather slot gi
                (0 if absent; padding rows are all-zero so gathered garbage
                in trimmed slots cannot leak).
      oh_d[m]:  [128, n_dense*ntile_d*128] same, per dense slot, with the
                row slot equal to the shard-local voxel index.
      counts:   [n_cores, n_gather] valid row count per gathered cell.
    """
    import ml_dtypes

    dense_chunks = sorted(dense_chunks)
    gathered = [c for c in range(n_chunks) if c not in dense_chunks]
    g_slot = np.full(n_chunks, -1, np.int64)
    for i, c in enumerate(gathered):
        g_slot[c] = i
    d_slot = np.full(n_chunks, -1, np.int64)
    for j, c in enumerate(dense_chunks):
        d_slot[c] = j
    n_gather = len(gathered)
    n_dense = len(dense_chunks)
    ntile_d = -(-vox_per_core // P)

    v2p = np.asarray(v2p_map).astype(np.int64)
    pidx = np.asarray(point_idx).astype(np.int64)
    seg = np.asarray(seg_ids).astype(np.int64)
    cidx = v2p[pidx]                      # composed voxel index per point
    core = cidx // vox_per_core
    lvox = cidx - core * vox_per_core
    chunk = seg >> 7                      # 128 segments per chunk
    segl = seg & 127
    cell = core * n_chunks + chunk        # [0, n_cores*n_chunks)
    key3 = (cell * vox_per_core + lvox) * 128 + segl
    uk, ucnt = np.unique(key3, return_counts=True)      # sorted pairs
    # fp8_e3m4 is exact for integers <= 15; overflow to bf16 in the
    # astronomically-unlikely case
    oh_is_fp8 = ucnt.max() <= 15
    oh_np_dt = ml_dtypes.float8_e3m4 if oh_is_fp8 else ml_dtypes.bfloat16
    pair_cell = uk // (vox_per_core * 128)
    pair_chunk = pair_cell % n_chunks
    pair_core = pair_cell // n_chunks
    pair_lvox = (uk // 128) % vox_per_core
    useg = (uk % 128).astype(np.int64)
    is_dense = d_slot[pair_chunk] >= 0

    # --- dense chunks: row slot == shard-local voxel index ---
    oh_d = None
    if n_dense:
        npd = ntile_d * P
        ohd_arr = np.zeros((n_cores * n_dense, npd, P), oh_np_dt)
        dm = is_dense
        dcell = pair_core[dm] * n_dense + d_slot[pair_chunk[dm]]
        ohd_arr[dcell, pair_lvox[dm], useg[dm]] = ucnt[dm].astype(np.float32)
        oh_d = []
        for m in range(n_cores):
            o = ohd_arr.reshape(n_cores, n_dense, ntile_d, P, P)[m]
            oh_d.append(np.ascontiguousarray(
                o.transpose(2, 0, 1, 3).reshape(P, n_dense * npd)))

    # --- gathered chunks: dedup rows, pack by in-cell slot ---
    gm = ~is_dense
    # gcell in [0, n_cores*n_gather) by gather slot
    gcell = pair_core[gm] * n_gather + g_slot[pair_chunk[gm]]
    rowkey = gcell * vox_per_core + pair_lvox[gm]
    urow, row_inv = np.unique(rowkey, return_inverse=True)
    rcell = (urow // vox_per_core).astype(np.int64)
    rvox = (urow % vox_per_core).astype(np.int64)
    counts = np.bincount(rcell, minlength=n_cores * n_gather)
    tiles_per_chunk = max(1, math.ceil(counts.max() / P))
    npc = tiles_per_chunk * P
    offs = np.concatenate([[0], np.cumsum(counts)])
    pos = np.arange(len(urow)) - offs[rcell]            # in-cell row slot
    vox_arr = np.full((n_cores * n_gather, npc), -1 if trim else 0, np.int16)
    vox_arr[rcell, pos] = rvox.astype(np.int16)
    oh = np.zeros((n_cores * n_gather, npc, P), oh_np_dt)
    pr_cell = rcell[row_inv]
    pr_pos = pos[row_inv]
    oh[pr_cell, pr_pos, useg[gm]] = ucnt[gm].astype(np.float32)
    idx16 = []
    ohs = []
    for m in range(n_cores):
        arr = vox_arr.reshape(n_cores, n_gather, npc)[m].reshape(-1, 16).T
        idx16.append(np.ascontiguousarray(np.tile(arr, (8, 1))))
        o = oh.reshape(n_cores, n_gather, tiles_per_chunk, P, P)[m]
        # [slot, tile, r, seg] -> [r, slot, tile, seg] -> [128, NT*128]
        ohs.append(np.ascontiguousarray(
            o.transpose(2, 0, 1, 3).reshape(P, n_gather * npc)))
    return idx16, ohs, oh_d, tiles_per_chunk, \
        counts.reshape(n_cores, n_gather), oh_is_fp8


def _batch_counts(counts, tiles_per_chunk, batch_tiles, trim=True,
                  force_full_chunks=0):
    """Per-(core, chunk, batch) valid index counts, clamped to the batch."""
    n_cores, n_chunks = counts.shape
    counts = counts.copy()
    if not trim:
        counts[:] = tiles_per_chunk * P
    counts[:, :force_full_chunks] = tiles_per_chunk * P
    n_batches = (tiles_per_chunk + batch_tiles - 1) // batch_tiles
    out = np.zeros((n_cores, n_chunks * n_batches), np.int32)
    for b in range(n_batches):
        start = b * batch_tiles * P
        width_tiles = min(batch_tiles, tiles_per_chunk - b * batch_tiles)
        cap = width_tiles * P
        vals = np.clip(counts - start, 0, cap)
        out[:, b::n_batches] = vals
    return out


def _build_nc(tiles_per_chunk, vox_per_core, D, S, n_cores, batch_tiles=8,
              main_reps=1, mode="full", single_core=False, gp_bufs=13,
              oh_bufs=3, acc_bufs=4, full_batches=None, need_memset=False,
              gather_dt="bf16", oh_is_fp8=True, dense_chunks=()):
    """mode: "full" | "nomm" (gathers only) | "nogather" (oh stream + matmul
    on constant data) | "noonehot" (gather + matmul, constant weights).
    main_reps repeats the main loop; with mode="full" the output stays
    correct (each rep recomputes the same sums; only the last is copied)."""
    import concourse.bacc as bacc
    import concourse.mybir as mybir
    import concourse.tile as tile
    from concourse.masks import make_identity

    f32 = mybir.dt.float32
    bf16 = mybir.dt.bfloat16
    f8e3 = mybir.dt.float8e3
    i16 = mybir.dt.int16
    i32 = mybir.dt.int32
    gdt = bf16 if gather_dt == "bf16" else f8e3
    odt = f8e3 if oh_is_fp8 else bf16
    n_chunks = S // P
    dense_chunks = sorted(dense_chunks)
    gathered = [c for c in range(n_chunks) if c not in dense_chunks]
    g_slot = {c: i for i, c in enumerate(gathered)}
    d_slot = {c: j for j, c in enumerate(dense_chunks)}
    n_gather = len(gathered)
    n_dense = len(dense_chunks)
    ntile_d = -(-vox_per_core // P)
    npd = ntile_d * P
    NT = n_gather * tiles_per_chunk            # total gathered point tiles
    NIDX = NT * P                              # total gathered rows
    npc = tiles_per_chunk * P
    # dense scatter-weight tables stream in pieces sized to the rotating
    # oh buffer (whose width serves both chunk kinds)
    oh_cols = max(npc, npd // 4 if n_dense else 0)
    piece_tiles = oh_cols // P
    if n_dense:
        # dense scatter-weight pieces rotate 4x faster than per-chunk tables
        oh_bufs = max(oh_bufs, 6)
    # multi-descriptor packets cut the per-row SDMA packet overhead of the
    # gather stream by ~25% (nomm slope 159us -> 118us)
    single_packet = os.environ.get("SINGLE_PACKET", "0") == "1"
    out_rows = S // n_cores                    # 256
    blk_tiles = out_rows // P                  # 2
    k_tiles = D // P                           # 2
    n_cols = 512                               # moving-operand tile width
    n_tiles_out = S // n_cols                  # 4

    nc = bacc.Bacc(
        "TRN2",
        target_bir_lowering=False,
        debug=False,
        enable_asserts=False,
        num_devices=n_cores,
        # SWDGE descriptor-ring carveout: must hold two in-flight
        # dma_gathers of batch_tiles*128 descriptors each.
        dynamic_dma_scratch_size=32768,
        # round-robin gathers over all 4 SWDGE queues: each queue's
        # descriptor generation runs on its own Q7 core pair.
        num_swdge_queues=4,
    )

    adapter = nc.dram_tensor("adapter", [vox_per_core, D], gdt,
                             kind="ExternalInput")
    idx16 = nc.dram_tensor("idx16", [P, NIDX // 16], i16, kind="ExternalInput")
    # host-pre-expanded scatter weights: per gather slot gi, tile t the lhsT
    # block [128 rows, 128 segs] lives at columns (gi*tpc + t)*128.
    ohd = nc.dram_tensor("ohd", [P, NT * P], odt, kind="ExternalInput")
    ohd_d = adapter_w = None
    if n_dense:
        ohd_d = nc.dram_tensor("ohd_d", [P, n_dense * npd], odt,
                               kind="ExternalInput")
        # the full shard, 128-partition-wrapped ([p, t, :] = voxel t*128+p),
        # fp8 so it stays SBUF-resident next to the gather buffers
        adapter_w = nc.dram_tensor("adapter_w", [P, ntile_d * D], f8e3,
                                   kind="ExternalInput")
    # caption embeds arrive L2-normalized from the host; only the transposed
    # copy is needed for the logits matmul.
    cet = nc.dram_tensor("cet", [D, S], bf16, kind="ExternalInput")
    lsr = nc.dram_tensor("lsr", [P, 1], f32, kind="ExternalInput")
    n_batches = (tiles_per_chunk + batch_tiles - 1) // batch_tiles
    if full_batches is None:
        full_batches = [False] * (n_gather * n_batches)
    cnts = nc.dram_tensor("cnts", [1, n_gather * n_batches], i32,
                          kind="ExternalInput")
    # bf16 logits halve the output-store DMA that sits on the post-RS2
    # critical path (host upcasts; ~0.4% rounding, far under the gate)
    out = nc.dram_tensor("logits_block", [out_rows, S], bf16,
                         kind="ExternalOutput")
    cc_in = nc.dram_tensor("cc_in", [S, D], f32, kind="Internal")
    half_rows = S // 2
    cc_out_h = [
        nc.dram_tensor(f"cc_out{h}", [half_rows // n_cores, D], f32,
                       kind="Internal")
        for h in range(2)
    ]

    with tile.TileContext(nc) as tc:
        with (
            tc.tile_pool(name="const", bufs=1) as constp,
            tc.tile_pool(name="gather", bufs=gp_bufs) as gp,
            tc.tile_pool(name="oh", bufs=oh_bufs) as ohp,
            tc.tile_pool(name="misc", bufs=1) as miscp,
            tc.tile_pool(name="fin", bufs=1) as finp,
            tc.tile_pool(name="fpsum", bufs=1, space="PSUM") as fpp,
        ):
            # ---- prologue loads, shortest-critical-path first ----
            # SP (sync) queue: batch counts + the first idx stripe unblock
            # the first gather within ~2 us.
            cnt_sb = constp.tile([1, n_gather * n_batches], i32)
            nc.sync.dma_start(cnt_sb[:], cnts.ap())
            # idx stripes are separate tiles so a gather only depends on the
            # stripe that covers its chunk (dep tracking is per-tile).
            chunk_cols = tiles_per_chunk * P // 16
            stripe_chunks = max(1, 2048 // chunk_cols)
            stripe_cols = stripe_chunks * chunk_cols
            stripe_bounds = []
            for s0 in range(0, NIDX // 16, stripe_cols):
                stripe_bounds.append((s0, min(s0 + stripe_cols, NIDX // 16)))
            idx_parts = [
                constp.tile([P, s1 - s0], i16, name=f"idx{s0}")
                for s0, s1 in stripe_bounds
            ]
            nc.sync.dma_start(idx_parts[0][:], idx16.ap()[:, : stripe_bounds[0][1]])
            ls_sb = finp.tile([P, 1], f32)
            nc.sync.dma_start(ls_sb[:], lsr.ap())
            # Later stripes aren't needed until chunk 4+; hint them behind
            # the first gathers so they don't hog the DMA engines up front.
            with tc.tile_wait_until(0.02):
                for (s0, s1), part in zip(stripe_bounds[1:], idx_parts[1:]):
                    nc.sync.dma_start(part[:], idx16.ap()[:, s0:s1])
            shard_sb = None
            if n_dense:
                # full fp8 shard, loaded contiguously once (no descriptors):
                # feeds every dense chunk's matmuls from SBUF
                shard_sb = miscp.tile([P, ntile_d, D], f8e3, name="shard")
                with tc.tile_wait_until(0.01):
                    nc.sync.dma_start(shard_sb[:], adapter_w.ap())
            els = finp.tile([P, 1], f32)
            nc.scalar.activation(els[:], ls_sb[:], mybir.ActivationFunctionType.Exp)
            ident = constp.tile([P, P], f32)
            make_identity(nc, ident[:])
            ident_w = constp.tile([P, P], odt)
            nc.vector.tensor_copy(out=ident_w[:], in_=ident[:])

            sums_sb = miscp.tile([P, n_chunks * D], f32)
            sq_scr = finp.tile([P, D], f32)

            # ACT queue: the transposed caption embeds (finale-only input).
            cet_sb = [finp.tile([P, S], bf16, tag=f"cet{k}", name=f"cet{k}")
                      for k in range(k_tiles)]
            for k in range(k_tiles):
                nc.scalar.dma_start(cet_sb[k][:], cet.ap()[k * P : (k + 1) * P, :])

            # ---- main: gather + scatter-weight matmul segment reduction ----
            # Chunk-staged pipeline: all of chunk c's rows are gathered into
            # one chunk-wide buffer (2 in flight), the chunk's scatter
            # weights stream in on the ACT HWDGE queue, and its matmuls run
            # as a single burst.
            if need_memset:
                for _slot in range(gp_bufs):
                    g_init = gp.tile([P, batch_tiles, D], gdt, tag="g",
                                     name="g_init")
                    nc.vector.memset(g_init[:], 0)
            g_static = None
            if mode == "nogather":
                g_static = miscp.tile([P, batch_tiles, D], gdt)
                nc.vector.memset(g_static[:], 1.0)
            if mode == "nomm":
                nc.vector.memset(sums_sb[:], 1.0)
            # shared register holding the full batch count: only batches that
            # are partial on some core pay a per-batch reg_load.
            vreg_full = None
            if any(full_batches) and mode != "nogather":
                vreg_full = nc.gpsimd.alloc_register()
                nc.gpsimd.reg_mov(vreg_full, batch_tiles * P)

            with tc.tile_pool(name="acc", bufs=acc_bufs, space="PSUM") as accp:
                for rep in range(main_reps):
                    for c in range(n_chunks):
                        dense = c in d_slot
                        acc = None
                        if mode != "nomm":
                            acc = accp.tile([P, D], f32, tag="acc", name="acc")
                        if dense:
                            if mode == "nomm":
                                continue
                            # dense chunk: matmuls read the SBUF-resident
                            # shard (no gather); scatter weights stream in
                            # oh-buffer-sized pieces
                            dj = d_slot[c]
                            t = 0
                            while t < ntile_d:
                                pt = min(piece_tiles, ntile_d - t)
                                piece = None
                                if mode != "noonehot":
                                    piece = ohp.tile([P, oh_cols], odt,
                                                     tag="oh", name="oh")
                                    col = (dj * ntile_d + t) * P
                                    nc.scalar.dma_start(
                                        piece[:, : pt * P],
                                        ohd_d.ap()[:, col : col + pt * P],
                                    )
                                for j in range(pt):
                                    oh_ap = (ident_w[:, :]
                                             if mode == "noonehot"
                                             else piece[:, j * P : (j + 1) * P])
                                    nc.tensor.matmul(
                                        acc[:],
                                        lhsT=oh_ap,
                                        rhs=shard_sb[:, t + j, :],
                                        start=(t + j == 0),
                                        stop=(t + j == ntile_d - 1),
                                    )
                                t += pt
                        else:
                            gi = g_slot[c]
                            oh_sb = None
                            if mode in ("full", "nogather"):
                                oh_sb = ohp.tile([P, oh_cols], odt,
                                                 tag="oh", name="oh")
                                nc.scalar.dma_start(
                                    oh_sb[:, :npc],
                                    ohd.ap()[:, gi * npc : (gi + 1) * npc],
                                )
                            gtiles = []
                            done = 0
                            while done < tiles_per_chunk:
                                bt = min(batch_tiles, tiles_per_chunk - done)
                                if mode == "nogather":
                                    done += bt
                                    continue
                                g = gp.tile([P, batch_tiles, D], gdt,
                                            tag="g", name="g")
                                gtiles.append(g)
                                col0 = (gi * tiles_per_chunk + done) * P // 16
                                nidx = bt * P
                                bidx = gi * n_batches + done // batch_tiles
                                part = idx_parts[col0 // stripe_cols]
                                pc0 = col0 % stripe_cols
                                if full_batches[bidx] and bt == batch_tiles:
                                    vreg = vreg_full
                                else:
                                    vreg = nc.gpsimd.alloc_register()
                                    nc.gpsimd.reg_load(
                                        vreg, cnt_sb[0:1, bidx : bidx + 1]
                                    )
                                nc.gpsimd.dma_gather(
                                    out_ap=g[:, :bt, :],
                                    in_ap=adapter.ap(),
                                    idxs_ap=part[:, pc0 : pc0 + nidx // 16],
                                    num_idxs=nidx,
                                    num_idxs_reg=vreg,
                                    elem_size=D,
                                    queue_num=bidx % 4,
                                    single_packet=single_packet,
                                )
                                if vreg is not vreg_full:
                                    nc.gpsimd.free_register(vreg)
                                done += bt
                            if mode == "nomm":
                                continue
                            for t in range(tiles_per_chunk):
                                if mode == "noonehot":
                                    oh_ap = ident_w[:, :]
                                else:
                                    oh_ap = oh_sb[:, t * P : (t + 1) * P]
                                rhs = (g_static[:, 0, :]
                                       if mode == "nogather"
                                       else gtiles[t // batch_tiles]
                                       [:, t % batch_tiles, :])
                                nc.tensor.matmul(
                                    acc[:],
                                    lhsT=oh_ap,
                                    rhs=rhs,
                                    start=(t == 0),
                                    stop=(t == tiles_per_chunk - 1),
                                )
                        if rep == main_reps - 1:
                            # evacuate PSUM on the ACT engine (its own SBUF
                            # port; keeps DVE idle so GpSimd never loses the
                            # shared port pair mid-gather-stream)
                            nc.scalar.activation(
                                sums_sb[:, c * D : (c + 1) * D],
                                acc[:],
                                mybir.ActivationFunctionType.Copy,
                            )
                            # stage this chunk's partial sums (ACT HWDGE queue
                            # so the SP queue stays free for other loads)
                            nc.scalar.dma_start(
                                cc_in.ap()[c * P : (c + 1) * P, :],
                                sums_sb[:, c * D : (c + 1) * D],
                            )
                            if c in (n_chunks // 2 - 1, n_chunks - 1):
                                h = 0 if c < n_chunks // 2 else 1
                                lo = h * half_rows
                                if single_core:
                                    nc.sync.dma_start(
                                        cc_out_h[h].ap(),
                                        cc_in.ap()[lo : lo + P, :],
                                    )
                                else:
                                    nc.gpsimd.collective_compute(
                                        "ReduceScatter",
                                        mybir.AluOpType.add,
                                        replica_groups=[list(range(n_cores))],
                                        ins=[cc_in.ap()[lo : lo + half_rows, :]],
                                        outs=[cc_out_h[h].ap()],
                                    )

            # ---- finale: per half-block normalize + logits rows ----
            # Pin the finale to the end of the schedule: without this the
            # tile scheduler hoists it into the middle of the main loop
            # (its collective input *can* be ready early), where it
            # head-of-line blocks the PE/DVE queues and stalls the gather
            # buffer recycling.
            finale_ctx = tc.tile_wait_until(0.3 * main_reps)
            finale_ctx.__enter__()
            pT = [finp.tile([P, out_rows], bf16, tag=f"pT{k}", name=f"pT{k}")
                  for k in range(k_tiles)]
            out_sb = [finp.tile([P, S], bf16, tag=f"os{m}", name=f"os{m}")
                      for m in range(blk_tiles)]
            for m in range(blk_tiles):
                blk = finp.tile([P, D], f32, tag=f"blk{m}", name=f"blk{m}")
                nc.sync.dma_start(blk[:], cc_out_h[m].ap())
                rs_inv = finp.tile([P, 1], f32, tag=f"ri{m}", name=f"ri{m}")
                nc.scalar.activation(
                    sq_scr[:],
                    blk[:],
                    mybir.ActivationFunctionType.Square,
                    accum_out=rs_inv[:],
                )
                nc.scalar.sqrt(rs_inv[:], rs_inv[:])
                nc.vector.tensor_scalar_max(rs_inv[:], rs_inv[:], 1e-12)
                nc.vector.reciprocal(rs_inv[:], rs_inv[:])
                nc.vector.tensor_tensor(
                    out=rs_inv[:], in0=rs_inv[:], in1=els[:],
                    op=mybir.AluOpType.mult,
                )
                nc.vector.tensor_scalar(
                    out=blk[:],
                    in0=blk[:],
                    scalar1=rs_inv[:],
                    scalar2=None,
                    op0=mybir.AluOpType.mult,
                )
                for k in range(k_tiles):
                    t_ps = fpp.tile([P, P], f32, tag="tps", bufs=1)
                    nc.tensor.transpose(
                        t_ps[:], blk[:, k * P : (k + 1) * P], ident[:]
                    )
                    nc.vector.tensor_copy(
                        out=pT[k][:, m * P : (m + 1) * P], in_=t_ps[:]
                    )
                for n in range(n_tiles_out):
                    o_ps = fpp.tile([P, n_cols], f32, tag="ops", bufs=2)
                    for k in range(k_tiles):
                        nc.tensor.matmul(
                            o_ps[:],
                            lhsT=pT[k][:, m * P : (m + 1) * P],
                            rhs=cet_sb[k][:, n * n_cols : (n + 1) * n_cols],
                            start=(k == 0),
                            stop=(k == k_tiles - 1),
                        )
                    nc.vector.tensor_copy(
                        out=out_sb[m][:, n * n_cols : (n + 1) * n_cols],
                        in_=o_ps[:],
                    )
                nc.sync.dma_start(
                    out.ap()[m * P : (m + 1) * P, :], out_sb[m][:]
                )
            finale_ctx.__exit__(None, None, None)
    nc.compile()
    return nc


def _prep(inputs_dict, n_cores, vox_per_core, D, S, batch_tiles=8):
    """Host preprocessing + derived build parameters, shared by kernel()
    and the benches."""
    gp_bufs = int(os.environ.get("GP_BUFS", "13"))
    oh_bufs = int(os.environ.get("OH_BUFS", "3"))
    dc_env = os.environ.get("DENSE_CHUNKS", DEFAULT_DENSE_CHUNKS)
    dense_chunks = tuple(
        int(x) for x in dc_env.split(",") if x.strip() != "")
    trim = True
    idx16, ohs, oh_d, tiles_per_chunk, counts, oh_is_fp8 = _preprocess(
        inputs_dict["v2p_map"], inputs_dict["point_idx"],
        inputs_dict["seg_ids"], n_cores, vox_per_core, S // P, trim=True,
        dense_chunks=dense_chunks,
    )
    # The first chunks covering all gp_bufs rotating buffers gather
    # untrimmed (pad idx 0 fetches a real row; its scatter weight is 0) so
    # every buffer is fully written with finite data before any trimmed
    # chunk can expose stale SBUF bytes.  Patch the pad slots of those
    # chunks from -1 to 0 in the wrapped idx16 layout.
    n_batches = -(-tiles_per_chunk // batch_tiles)
    ffc = min(-(-gp_bufs // n_batches) + 1, S // P)
    npc16 = tiles_per_chunk * P // 16
    for m in range(n_cores):
        head = idx16[m][:, : ffc * npc16]
        head[head < 0] = 0
    # a zero-valid-count gather would emit no descriptors and never fire its
    # completion semaphore; fall back to untrimmed padding in that case
    if _batch_counts(counts, tiles_per_chunk, batch_tiles, trim=True,
                     force_full_chunks=ffc).min() == 0:
        trim = False
        ffc = 0
        idx16, ohs, oh_d, tiles_per_chunk, counts, oh_is_fp8 = _preprocess(
            inputs_dict["v2p_map"], inputs_dict["point_idx"],
            inputs_dict["seg_ids"], n_cores, vox_per_core, S // P, trim=False,
            dense_chunks=dense_chunks,
        )
    need_memset = False
    bc = _batch_counts(counts, tiles_per_chunk, batch_tiles, trim=trim,
                       force_full_chunks=ffc)
    full_batches = (bc.min(axis=0) == _batch_counts(
        np.full_like(counts, tiles_per_chunk * P), tiles_per_chunk,
        batch_tiles).min(axis=0)).tolist()
    # fp8_e3m4 rows: 256 B descriptors, half the random-read HBM bytes of
    # bf16; quantization (RMS ~1.5%) averages out over ~460-row segment
    # means and lands well under the 2e-2 gate (measured 1.23e-2)
    gather_dt = os.environ.get("GATHER_DT", "fp8")
    return dict(
        idx16=idx16, ohs=ohs, oh_d=oh_d, dense_chunks=dense_chunks,
        tiles_per_chunk=tiles_per_chunk, counts=counts,
        oh_is_fp8=oh_is_fp8, trim=trim, need_memset=need_memset, ffc=ffc,
        full_batches=full_batches, gp_bufs=gp_bufs, oh_bufs=oh_bufs,
        batch_tiles=batch_tiles,
        gather_dt=gather_dt, n_cores=n_cores, vox_per_core=vox_per_core,
        D=D, S=S,
    )


def _nc_from_prep(prep, main_reps=1, mode="full"):
    return _build_nc(
        prep["tiles_per_chunk"], prep["vox_per_core"], prep["D"], prep["S"],
        prep["n_cores"], batch_tiles=prep["batch_tiles"],
        main_reps=main_reps, mode=mode, gp_bufs=prep["gp_bufs"],
        oh_bufs=prep["oh_bufs"],
        full_batches=prep["full_batches"], need_memset=prep["need_memset"],
        gather_dt=prep["gather_dt"], oh_is_fp8=prep["oh_is_fp8"],
        dense_chunks=prep["dense_chunks"],
        single_core=os.environ.get("SINGLE_CORE", "0") == "1",
    )


def _make_in_maps(prep, inputs_dict):
    import ml_dtypes

    bf = ml_dtypes.bfloat16
    gather_np_dt = bf if prep["gather_dt"] == "bf16" else ml_dtypes.float8_e3m4
    n_cores = prep["n_cores"]
    vox_per_core = prep["vox_per_core"]
    af32 = np.asarray(inputs_dict["adapter_feats"], np.float32)
    af = np.ascontiguousarray(af32.astype(gather_np_dt))
    ce_f32 = np.asarray(inputs_dict["caption_embed"], np.float32)
    ce_n = ce_f32 / np.clip(
        np.linalg.norm(ce_f32, axis=-1, keepdims=True), 1e-12, None
    )
    cet_np = np.ascontiguousarray(ce_n.T.astype(bf))
    ls = np.asarray(inputs_dict["logit_scale"], np.float32).reshape(-1)[0]
    ls_rep = np.full((P, 1), ls, np.float32)
    bc = _batch_counts(prep["counts"], prep["tiles_per_chunk"],
                       prep["batch_tiles"], trim=prep["trim"],
                       force_full_chunks=prep["ffc"])
    n_dense = len(prep["dense_chunks"])
    if n_dense:
        ntile_d = -(-vox_per_core // P)
        af8 = af32.astype(ml_dtypes.float8_e3m4)
        af8_pad = np.zeros((n_cores, ntile_d * P, D_FULL), af8.dtype)
        af8_pad[:, : vox_per_core] = af8.reshape(n_cores, vox_per_core, -1)
        # 128-partition-wrapped: [p, t*D + d] = voxel t*128+p
        af_w = np.ascontiguousarray(
            af8_pad.reshape(n_cores, ntile_d, P, D_FULL)
            .transpose(0, 2, 1, 3).reshape(n_cores, P, ntile_d * D_FULL))
    in_maps = []
    for m in range(n_cores):
        im = {
            "adapter": af[m * vox_per_core : (m + 1) * vox_per_core],
            "idx16": prep["idx16"][m],
            "ohd": prep["ohs"][m],
            "cet": cet_np,
            "lsr": ls_rep,
            "cnts": bc[m : m + 1],
        }
        if n_dense:
            im["ohd_d"] = prep["oh_d"][m]
            im["adapter_w"] = af_w[m]
        in_maps.append(im)
    return in_maps


def _run(inputs_dict, n_cores, vox_per_core, D, S, batch_tiles=8, trace=False):
    from concourse.bass_utils import run_bass_kernel_spmd

    prep = _prep(inputs_dict, n_cores, vox_per_core, D, S,
                 batch_tiles=batch_tiles)
    nc = _nc_from_prep(prep, main_reps=1, mode="full")
    in_maps = _make_in_maps(prep, inputs_dict)
    res = run_bass_kernel_spmd(
        nc, in_maps, core_ids=list(range(n_cores)), trace=trace
    )
    blocks = [res.results[m]["logits_block"] for m in range(n_cores)]
    return _assemble(blocks, S, n_cores), res


def _assemble(blocks, S, n_cores):
    """Core r's output block holds segment rows for chunk r (tile 0) and
    chunk n_cores+r (tile 1)."""
    half = S // 2
    full = np.empty((S, blocks[0].shape[1]), np.float32)
    for r in range(n_cores):
        full[r * P : (r + 1) * P] = blocks[r][:P]
        full[half + r * P : half + (r + 1) * P] = blocks[r][P : 2 * P]
    return full


def kernel(adapter_feats, caption_embed, logit_scale, v2p_map, point_idx,
           seg_ids, num_segments=S_FULL, **_):
    logits, _res = _run(
        {
            "adapter_feats": adapter_feats,
            "caption_embed": caption_embed,
            "logit_scale": logit_scale,
            "v2p_map": v2p_map,
            "point_idx": point_idx,
            "seg_ids": seg_ids,
        },
        N_CORES,
        N_VOX // N_CORES,
        D_FULL,
        S_FULL,
    )
    return logits
